# revision 5
# baseline (speedup 1.0000x reference)
"""MoE (top-2 of 8 experts, SwiGLU) kernel for 8 TRN2 NeuronCores.

Strategy: expert-parallel. Core e holds expert e's weights and computes the
dense masked formulation for ALL tokens: y_e = (silu(hs@Wg_e) * (hs@Wu_e)) @ Wd_e,
scaled per-token by the combine weight w[e,t] (zero if token t is not routed to
expert e). The weighted partials are summed across the 8 cores with per-block
ReduceScatter(add); each core ends with a 1/8 slice of the output.

Everything runs in "transposed land": the moving matmul operand is always the
token axis, so hidden_states is shipped as hsT=[H,T] and the kernel produces
yT=[H,T] shards; the host transposes back at unshard time.

Matmul operands are bf16 (fp32 PSUM accumulation) — rel err vs the fp32
reference is ~1e-3, far inside tolerance, and bf16 runs the PE at full rate.
"""

import numpy as np
import ml_dtypes

import jax
import concourse.bass as bass
import concourse.tile as tile
from concourse import bacc, mybir
from concourse.bass import ts

E, H, I, T, KTOP = 8, 2048, 1408, 4096, 2
NCORES = 8
TB = 512  # tokens per block (= max f32 PSUM free dim)

BF16 = mybir.dt.bfloat16
F32 = mybir.dt.float32


def _build_moe(h=H, i_sz=I, t=T, tb=TB, ncores=NCORES):
    """Per-core SPMD graph. Inputs (per core e):
    hsT [h,t] bf16 (replicated), wg [h,i] bf16, wu [h,i] bf16, wd [i,h] bf16,
    wc [1,t] f32 (combine weights for this core's expert).
    Output: out [ntb, h//ncores, tb] f32 — this core's ReduceScatter shards.
    """
    hc, ic, ntb = h // 128, i_sz // 128, t // tb
    shard = h // ncores
    nc = bacc.Bacc("TRN2", target_bir_lowering=False, debug=False,
                   num_devices=ncores)

    hsT = nc.declare_dram_parameter("hsT", [h, t], BF16, isOutput=False).ap()
    wg = nc.declare_dram_parameter("wg", [h, i_sz], BF16, isOutput=False).ap()
    wu = nc.declare_dram_parameter("wu", [h, i_sz], BF16, isOutput=False).ap()
    wd = nc.declare_dram_parameter("wd", [i_sz, h], BF16, isOutput=False).ap()
    wc = nc.declare_dram_parameter("wc", [1, t], F32, isOutput=False).ap()
    out = nc.declare_dram_parameter("out", [ntb, shard, tb], F32,
                                    isOutput=True).ap()

    silu = mybir.ActivationFunctionType.Sigmoid
    rgroups = [list(range(ncores))]

    with tile.TileContext(nc) as tc:
        with (
            tc.tile_pool(name="wpool", bufs=1) as wpool,
            tc.tile_pool(name="hspool", bufs=2) as hspool,
            tc.tile_pool(name="apool", bufs=1) as apool,
            tc.tile_pool(name="stage", bufs=3) as stage,
            tc.tile_pool(name="pg", bufs=2, space="PSUM") as pg,
            tc.tile_pool(name="pu", bufs=2, space="PSUM") as pu,
            tc.tile_pool(name="py", bufs=2, space="PSUM") as py,
            tc.tile_pool(name="dram", bufs=1, space="DRAM") as dram,
        ):
            # Resident weights, laid out [128, chunk, free] so that
            # [:, c, ts(j,128)] is a ready [K=128, M=128] stationary operand.
            wg_sb = wpool.tile([128, hc, i_sz], BF16)
            nc.sync.dma_start(out=wg_sb[:], in_=wg.rearrange("(c p) i -> p c i", p=128))
            wu_sb = wpool.tile([128, hc, i_sz], BF16)
            nc.sync.dma_start(out=wu_sb[:], in_=wu.rearrange("(c p) i -> p c i", p=128))
            wd_sb = wpool.tile([128, ic, h], BF16)
            nc.sync.dma_start(out=wd_sb[:], in_=wd.rearrange("(c p) j -> p c j", p=128))

            # Combine weights broadcast across all 128 partitions.
            wc_sb = wpool.tile([128, t], F32)
            nc.sync.dma_start(out=wc_sb[:], in_=wc.broadcast_to([128, t]))

            pT = dram.tile([ntb, h, tb], F32)
            rs = dram.tile([ntb, shard, tb], F32)

            for b in range(ntb):
                hs_t = hspool.tile([128, hc, tb], BF16)
                nc.sync.dma_start(
                    out=hs_t[:],
                    in_=hsT[:, ts(b, tb)].rearrange("(c p) t -> p c t", p=128))

                a_sb = apool.tile([128, ic, tb], BF16)
                for it in range(ic):
                    psg = pg.tile([128, tb], F32)
                    psu = pu.tile([128, tb], F32)
                    for c in range(hc):
                        nc.tensor.matmul(psg[:], lhsT=wg_sb[:, c, ts(it, 128)],
                                         rhs=hs_t[:, c, :],
                                         start=(c == 0), stop=(c == hc - 1))
                    for c in range(hc):
                        nc.tensor.matmul(psu[:], lhsT=wu_sb[:, c, ts(it, 128)],
                                         rhs=hs_t[:, c, :],
                                         start=(c == 0), stop=(c == hc - 1))
                    sil = stage.tile([128, tb], F32, tag="sil")
                    nc.scalar.activation(out=sil[:], in_=psg[:], func=silu)
                    nc.vector.tensor_mul(sil[:], sil[:], psg[:])
                    nc.vector.tensor_mul(a_sb[:, it, :], sil[:], psu[:])

                for ht in range(hc):
                    psy = py.tile([128, tb], F32)
                    for c2 in range(ic):
                        nc.tensor.matmul(psy[:], lhsT=wd_sb[:, c2, ts(ht, 128)],
                                         rhs=a_sb[:, c2, :],
                                         start=(c2 == 0), stop=(c2 == ic - 1))
                    po = stage.tile([128, tb], F32, tag="pout")
                    nc.vector.tensor_mul(po[:], psy[:], wc_sb[:, ts(b, tb)])
                    nc.sync.dma_start(out=pT[b, ts(ht, 128), :], in_=po[:])

                nc.gpsimd.collective_compute(
                    "ReduceScatter", mybir.AluOpType.add,
                    replica_groups=rgroups,
                    ins=[pT[b].opt()], outs=[rs[b].opt()])
                nc.sync.dma_start(out=out[b], in_=rs[b])

    nc.compile()
    return nc


class _Runner:
    """Compile once, execute many. Mirrors bass2jax.run_bass_via_pjrt's
    multi-core path but keeps the jitted callable (and device-resident
    inputs) alive so repeat executions skip XLA/NEFF compilation."""

    def __init__(self, nc, n_cores):
        from concourse import bass2jax
        from jax.experimental.shard_map import shard_map
        from jax.sharding import Mesh, PartitionSpec

        bass2jax.install_neuronx_cc_hook()
        assert nc.partition_id_tensor is None or True
        partition_name = (nc.partition_id_tensor.name
                          if nc.partition_id_tensor else None)

        in_names, out_names, out_avals, zero_outs = [], [], [], []
        for alloc in nc.m.functions[0].allocations:
            if not isinstance(alloc, mybir.MemoryLocationSet):
                continue
            name = alloc.memorylocations[0].name
            if alloc.kind == "ExternalInput":
                if name != partition_name:
                    in_names.append(name)
            elif alloc.kind == "ExternalOutput":
                shape = tuple(alloc.tensor_shape)
                dtype = mybir.dt.np(alloc.dtype)
                out_names.append(name)
                out_avals.append(jax.core.ShapedArray(shape, dtype))
                zero_outs.append(np.zeros(shape, dtype))
        self.n_params = len(in_names)
        self.param_names = list(in_names)
        self.out_names = out_names
        self.out_avals = out_avals
        self.n_cores = n_cores
        all_names = in_names + out_names
        if partition_name is not None:
            all_names.append(partition_name)

        def _body(*args):
            operands = list(args)
            if partition_name is not None:
                operands.append(bass2jax.partition_id_tensor())
            outs = bass2jax._bass_exec_p.bind(
                *operands,
                out_avals=tuple(out_avals),
                in_names=tuple(all_names),
                out_names=tuple(out_names),
                lowering_input_output_aliases=(),
                sim_require_finite=True,
                sim_require_nnan=True,
                nc=nc,
            )
            return tuple(outs)

        devices = jax.devices()[:n_cores]
        assert len(devices) == n_cores
        mesh = Mesh(np.asarray(devices), ("core",))
        n_ops = self.n_params + len(out_names)
        self._fn = jax.jit(
            shard_map(_body, mesh=mesh,
                      in_specs=(PartitionSpec("core"),) * n_ops,
                      out_specs=(PartitionSpec("core"),) * len(out_names),
                      check_rep=False),
            keep_unused=True)
        self._zeros = [
            np.zeros((n_cores * z.shape[0], *z.shape[1:]), z.dtype)
            for z in zero_outs
        ]
        self._dev_args = None

    def prepare(self, in_maps):
        """Stage concatenated inputs (host->device happens on first exec)."""
        concat = [
            np.concatenate([np.asarray(in_maps[c][name])
                            for c in range(self.n_cores)], axis=0)
            for name in self.param_names
        ]
        self._dev_args = [jax.device_put(a) for a in concat + self._zeros]

    def execute(self):
        outs = self._fn(*self._dev_args)
        jax.block_until_ready(outs)
        return outs

    def run(self, in_maps):
        self.prepare(in_maps)
        outs = self.execute()
        return [
            {name: np.asarray(outs[i]).reshape(self.n_cores,
                                               *self.out_avals[i].shape)[c]
             for i, name in enumerate(self.out_names)}
            for c in range(self.n_cores)
        ]


_RUNNER = None


def _get_runner():
    global _RUNNER
    if _RUNNER is None:
        nc = _build_moe()
        _RUNNER = _Runner(nc, NCORES)
    return _RUNNER


def _prep_in_maps(hidden_states, top_k_index, top_k_weights, Wg, Wu, Wd):
    hs = np.ascontiguousarray(np.asarray(hidden_states, dtype=np.float32))
    idx = np.asarray(top_k_index).astype(np.int64)
    tw = np.asarray(top_k_weights, dtype=np.float32)

    # Combine weights w[e,t] = sum_k tw[t,k] * [idx[t,k]==e]  (host: O(T*K))
    w = np.zeros((E, T), dtype=np.float32)
    tarange = np.arange(T)
    for k in range(KTOP):
        np.add.at(w, (idx[:, k], tarange), tw[:, k])

    hsT = np.ascontiguousarray(hs.T).astype(ml_dtypes.bfloat16)
    in_maps = []
    for e in range(NCORES):
        in_maps.append({
            "hsT": hsT,
            "wg": np.asarray(Wg[e], dtype=np.float32).astype(ml_dtypes.bfloat16),
            "wu": np.asarray(Wu[e], dtype=np.float32).astype(ml_dtypes.bfloat16),
            "wd": np.asarray(Wd[e], dtype=np.float32).astype(ml_dtypes.bfloat16),
            "wc": w[e:e + 1],
        })
    return in_maps


def _assemble(results):
    ntb, shard = T // TB, H // NCORES
    yT = np.empty((H, T), dtype=np.float32)
    for c in range(NCORES):
        r = results[c]["out"]  # [ntb, shard, TB]
        for b in range(ntb):
            yT[c * shard:(c + 1) * shard, b * TB:(b + 1) * TB] = r[b]
    return np.ascontiguousarray(yT.T)


def kernel(hidden_states, top_k_index, top_k_weights, Wg, Wu, Wd):
    runner = _get_runner()
    in_maps = _prep_in_maps(hidden_states, top_k_index, top_k_weights,
                            Wg, Wu, Wd)
    results = runner.run(in_maps)
    return _assemble(results)


# revision 8
# speedup vs baseline: 3.3062x; 3.3062x over previous
"""MoE (top-2 of 8 experts, SwiGLU) kernel for 8 TRN2 NeuronCores.

Strategy: expert-parallel. Core e holds expert e's weights and computes the
dense masked formulation for ALL tokens: y_e = (silu(hs@Wg_e) * (hs@Wu_e)) @ Wd_e,
scaled per-token by the combine weight w[e,t] (zero if token t is not routed to
expert e). The weighted partials are summed across the 8 cores with per-block
ReduceScatter(add); each core ends with a 1/8 slice of the output.

Everything runs in "transposed land": the moving matmul operand is always the
token axis, so hidden_states is shipped as hsT=[H,T] and the kernel produces
yT=[H,T] shards; the host transposes back at unshard time.

Matmul operands are bf16 (fp32 PSUM accumulation) — rel err vs the fp32
reference is ~1e-3, far inside tolerance, and bf16 runs the PE at full rate.
"""

import numpy as np
import ml_dtypes

import jax
import concourse.bass as bass
import concourse.tile as tile
from concourse import bacc, mybir
from concourse.bass import ts

E, H, I, T, KTOP = 8, 2048, 1408, 4096, 2
NCORES = 8
TB = 512  # tokens per block (= max f32 PSUM free dim)

BF16 = mybir.dt.bfloat16
F32 = mybir.dt.float32


def _build_moe(h=H, i_sz=I, t=T, tb=TB, ncores=NCORES):
    """Per-core SPMD graph. Inputs (per core e):
    hsT [h,t] bf16 (replicated), wg [h,i] bf16, wu [h,i] bf16, wd [i,h] bf16,
    wc [1,t] f32 (combine weights for this core's expert).
    Output: out [ntb, h//ncores, tb] f32 — this core's ReduceScatter shards.
    """
    hc, ic, ntb = h // 128, i_sz // 128, t // tb
    shard = h // ncores
    nc = bacc.Bacc("TRN2", target_bir_lowering=False, debug=False,
                   num_devices=ncores)

    hsT = nc.declare_dram_parameter("hsT", [h, t], BF16, isOutput=False).ap()
    wg = nc.declare_dram_parameter("wg", [h, i_sz], BF16, isOutput=False).ap()
    wu = nc.declare_dram_parameter("wu", [h, i_sz], BF16, isOutput=False).ap()
    wd = nc.declare_dram_parameter("wd", [i_sz, h], BF16, isOutput=False).ap()
    wc = nc.declare_dram_parameter("wc", [1, t], F32, isOutput=False).ap()
    out = nc.declare_dram_parameter("out", [ntb, shard, tb], F32,
                                    isOutput=True).ap()

    silu = mybir.ActivationFunctionType.Sigmoid
    rgroups = [list(range(ncores))]

    with tile.TileContext(nc) as tc:
        with (
            tc.tile_pool(name="wpool", bufs=1) as wpool,
            tc.tile_pool(name="hspool", bufs=2) as hspool,
            tc.tile_pool(name="apool", bufs=1) as apool,
            tc.tile_pool(name="stage", bufs=3) as stage,
            tc.tile_pool(name="pg", bufs=2, space="PSUM") as pg,
            tc.tile_pool(name="pu", bufs=2, space="PSUM") as pu,
            tc.tile_pool(name="py", bufs=2, space="PSUM") as py,
            tc.tile_pool(name="dram", bufs=1, space="DRAM") as dram,
        ):
            # Resident weights, laid out [128, chunk, free] so that
            # [:, c, ts(j,128)] is a ready [K=128, M=128] stationary operand.
            wg_sb = wpool.tile([128, hc, i_sz], BF16)
            nc.sync.dma_start(out=wg_sb[:], in_=wg.rearrange("(c p) i -> p c i", p=128))
            wu_sb = wpool.tile([128, hc, i_sz], BF16)
            nc.sync.dma_start(out=wu_sb[:], in_=wu.rearrange("(c p) i -> p c i", p=128))
            wd_sb = wpool.tile([128, ic, h], BF16)
            nc.sync.dma_start(out=wd_sb[:], in_=wd.rearrange("(c p) j -> p c j", p=128))

            # Combine weights broadcast across all 128 partitions.
            wc_sb = wpool.tile([128, t], F32)
            nc.sync.dma_start(out=wc_sb[:], in_=wc.broadcast_to([128, t]))

            pT = dram.tile([ntb, h, tb], F32)
            rs = dram.tile([ntb, shard, tb], F32)

            for b in range(ntb):
                hs_t = hspool.tile([128, hc, tb], BF16)
                nc.sync.dma_start(
                    out=hs_t[:],
                    in_=hsT[:, ts(b, tb)].rearrange("(c p) t -> p c t", p=128))

                a_sb = apool.tile([128, ic, tb], BF16)
                for it in range(ic):
                    psg = pg.tile([128, tb], F32)
                    psu = pu.tile([128, tb], F32)
                    for c in range(hc):
                        nc.tensor.matmul(psg[:], lhsT=wg_sb[:, c, ts(it, 128)],
                                         rhs=hs_t[:, c, :],
                                         start=(c == 0), stop=(c == hc - 1))
                    for c in range(hc):
                        nc.tensor.matmul(psu[:], lhsT=wu_sb[:, c, ts(it, 128)],
                                         rhs=hs_t[:, c, :],
                                         start=(c == 0), stop=(c == hc - 1))
                    sil = stage.tile([128, tb], F32, tag="sil")
                    nc.scalar.activation(out=sil[:], in_=psg[:], func=silu)
                    nc.vector.tensor_mul(sil[:], sil[:], psg[:])
                    nc.vector.tensor_mul(a_sb[:, it, :], sil[:], psu[:])

                for ht in range(hc):
                    psy = py.tile([128, tb], F32)
                    for c2 in range(ic):
                        nc.tensor.matmul(psy[:], lhsT=wd_sb[:, c2, ts(ht, 128)],
                                         rhs=a_sb[:, c2, :],
                                         start=(c2 == 0), stop=(c2 == ic - 1))
                    po = stage.tile([128, tb], F32, tag="pout")
                    nc.vector.tensor_mul(po[:], psy[:], wc_sb[:, ts(b, tb)])
                    nc.sync.dma_start(out=pT[b, ts(ht, 128), :], in_=po[:])

                nc.gpsimd.collective_compute(
                    "ReduceScatter", mybir.AluOpType.add,
                    replica_groups=rgroups,
                    ins=[pT[b].opt()], outs=[rs[b].opt()])
                nc.sync.dma_start(out=out[b], in_=rs[b])

    nc.compile()
    return nc


class _Runner:
    """Compile once, execute many. Mirrors bass2jax.run_bass_via_pjrt's
    multi-core path but keeps the jitted callable (and device-resident
    inputs) alive so repeat executions skip XLA/NEFF compilation."""

    def __init__(self, nc, n_cores):
        from concourse import bass2jax
        from jax.experimental.shard_map import shard_map
        from jax.sharding import Mesh, PartitionSpec

        bass2jax.install_neuronx_cc_hook()
        assert nc.partition_id_tensor is None or True
        partition_name = (nc.partition_id_tensor.name
                          if nc.partition_id_tensor else None)

        in_names, out_names, out_avals, zero_outs = [], [], [], []
        for alloc in nc.m.functions[0].allocations:
            if not isinstance(alloc, mybir.MemoryLocationSet):
                continue
            name = alloc.memorylocations[0].name
            if alloc.kind == "ExternalInput":
                if name != partition_name:
                    in_names.append(name)
            elif alloc.kind == "ExternalOutput":
                shape = tuple(alloc.tensor_shape)
                dtype = mybir.dt.np(alloc.dtype)
                out_names.append(name)
                out_avals.append(jax.core.ShapedArray(shape, dtype))
                zero_outs.append(np.zeros(shape, dtype))
        self.n_params = len(in_names)
        self.param_names = list(in_names)
        self.out_names = out_names
        self.out_avals = out_avals
        self.n_cores = n_cores
        all_names = in_names + out_names
        if partition_name is not None:
            all_names.append(partition_name)

        def _body(*args):
            operands = list(args)
            if partition_name is not None:
                operands.append(bass2jax.partition_id_tensor())
            outs = bass2jax._bass_exec_p.bind(
                *operands,
                out_avals=tuple(out_avals),
                in_names=tuple(all_names),
                out_names=tuple(out_names),
                lowering_input_output_aliases=(),
                sim_require_finite=True,
                sim_require_nnan=True,
                nc=nc,
            )
            return tuple(outs)

        devices = jax.devices()[:n_cores]
        assert len(devices) == n_cores
        mesh = Mesh(np.asarray(devices), ("core",))
        n_ops = self.n_params + len(out_names)
        self._body = _body
        self._mesh = mesh
        self._in_specs = (PartitionSpec("core"),) * n_ops
        self._out_specs = (PartitionSpec("core"),) * len(out_names)
        self._shard_map = shard_map
        self._fn = jax.jit(
            shard_map(_body, mesh=mesh,
                      in_specs=self._in_specs,
                      out_specs=self._out_specs,
                      check_rep=False),
            keep_unused=True)
        self._chains = {}
        self._zeros = [
            np.zeros((n_cores * z.shape[0], *z.shape[1:]), z.dtype)
            for z in zero_outs
        ]
        self._dev_args = None

    def prepare(self, in_maps):
        """Stage concatenated inputs (host->device happens on first exec)."""
        concat = [
            np.concatenate([np.asarray(in_maps[c][name])
                            for c in range(self.n_cores)], axis=0)
            for name in self.param_names
        ]
        self._dev_args = [jax.device_put(a) for a in concat + self._zeros]

    def execute(self):
        outs = self._fn(*self._dev_args)
        jax.block_until_ready(outs)
        return outs

    def execute_chain(self, k):
        """Issue k async executions back-to-back, block once at the end.
        Device-side queuing overlaps the per-dispatch host round-trip, so
        wall(k) ≈ floor + k * hw_exec and the slope over k isolates
        hw_exec."""
        outs = None
        for _ in range(k):
            outs = self._fn(*self._dev_args)
        jax.block_until_ready(outs)
        return outs

    def run(self, in_maps):
        self.prepare(in_maps)
        outs = self.execute()
        return [
            {name: np.asarray(outs[i]).reshape(self.n_cores,
                                               *self.out_avals[i].shape)[c]
             for i, name in enumerate(self.out_names)}
            for c in range(self.n_cores)
        ]


_RUNNER = None


def _get_runner():
    global _RUNNER
    if _RUNNER is None:
        nc = _build_moe()
        _RUNNER = _Runner(nc, NCORES)
    return _RUNNER


def _prep_in_maps(hidden_states, top_k_index, top_k_weights, Wg, Wu, Wd):
    hs = np.ascontiguousarray(np.asarray(hidden_states, dtype=np.float32))
    idx = np.asarray(top_k_index).astype(np.int64)
    tw = np.asarray(top_k_weights, dtype=np.float32)

    # Combine weights w[e,t] = sum_k tw[t,k] * [idx[t,k]==e]  (host: O(T*K))
    w = np.zeros((E, T), dtype=np.float32)
    tarange = np.arange(T)
    for k in range(KTOP):
        np.add.at(w, (idx[:, k], tarange), tw[:, k])

    hsT = np.ascontiguousarray(hs.T).astype(ml_dtypes.bfloat16)
    in_maps = []
    for e in range(NCORES):
        in_maps.append({
            "hsT": hsT,
            "wg": np.asarray(Wg[e], dtype=np.float32).astype(ml_dtypes.bfloat16),
            "wu": np.asarray(Wu[e], dtype=np.float32).astype(ml_dtypes.bfloat16),
            "wd": np.asarray(Wd[e], dtype=np.float32).astype(ml_dtypes.bfloat16),
            "wc": w[e:e + 1],
        })
    return in_maps


def _assemble(results):
    ntb, shard = T // TB, H // NCORES
    yT = np.empty((H, T), dtype=np.float32)
    for c in range(NCORES):
        r = results[c]["out"]  # [ntb, shard, TB]
        for b in range(ntb):
            yT[c * shard:(c + 1) * shard, b * TB:(b + 1) * TB] = r[b]
    return np.ascontiguousarray(yT.T)


def kernel(hidden_states, top_k_index, top_k_weights, Wg, Wu, Wd):
    runner = _get_runner()
    in_maps = _prep_in_maps(hidden_states, top_k_index, top_k_weights,
                            Wg, Wu, Wd)
    results = runner.run(in_maps)
    return _assemble(results)


# revision 21
# speedup vs baseline: 81.9278x; 24.7799x over previous
"""MoE (top-2 of 8 experts, SwiGLU) kernel for 8 TRN2 NeuronCores.

Strategy: expert-parallel. Core e holds expert e's weights and computes the
dense masked formulation for ALL tokens: y_e = (silu(hs@Wg_e) * (hs@Wu_e)) @ Wd_e,
scaled per-token by the combine weight w[e,t] (zero if token t is not routed to
expert e). The weighted partials are summed across the 8 cores with per-block
ReduceScatter(add); each core ends with a 1/8 slice of the output.

Everything runs in "transposed land": the moving matmul operand is always the
token axis, so hidden_states is shipped as hsT=[H,T] and the kernel produces
yT=[H,T] shards; the host transposes back at unshard time.

Matmul operands are bf16 (fp32 PSUM accumulation) — rel err vs the fp32
reference is ~1e-3, far inside tolerance, and bf16 runs the PE at full rate.
"""

import numpy as np
import ml_dtypes

import jax
import concourse.bass as bass
import concourse.tile as tile
from concourse import bacc, mybir
from concourse.bass import ts

E, H, I, T, KTOP = 8, 2048, 1408, 4096, 2
NCORES = 8
TB = 512  # tokens per block (= max f32 PSUM free dim)

BF16 = mybir.dt.bfloat16
F32 = mybir.dt.float32


def _build_moe(h=H, i_sz=I, t=T, tb=TB, ncores=NCORES, use_collective=True):
    """Per-core SPMD graph. Inputs (per core e):
    hsT [h,t] bf16 (replicated), wg [h,i] bf16, wu [h,i] bf16, wd [i,h] bf16,
    wc [1,t] f32 (combine weights for this core's expert).
    Output: out [ntb, h//ncores, tb] f32 — this core's ReduceScatter shards.
    """
    hc, ic, ntb = h // 128, i_sz // 128, t // tb
    shard = h // ncores
    nc = bacc.Bacc("TRN2", target_bir_lowering=False, debug=False,
                   num_devices=ncores)

    hsT = nc.declare_dram_parameter("hsT", [h, t], BF16, isOutput=False).ap()
    wg = nc.declare_dram_parameter("wg", [h, i_sz], BF16, isOutput=False).ap()
    wu = nc.declare_dram_parameter("wu", [h, i_sz], BF16, isOutput=False).ap()
    wd = nc.declare_dram_parameter("wd", [i_sz, h], BF16, isOutput=False).ap()
    wc = nc.declare_dram_parameter("wc", [1, t], F32, isOutput=False).ap()
    out = nc.declare_dram_parameter("out", [ntb, shard, tb], F32,
                                    isOutput=True).ap()

    silu = mybir.ActivationFunctionType.Sigmoid
    rgroups = [list(range(ncores))]

    with tile.TileContext(nc) as tc:
        with (
            tc.tile_pool(name="wpool", bufs=1) as wpool,
            tc.tile_pool(name="hspool", bufs=2) as hspool,
            tc.tile_pool(name="apool", bufs=1) as apool,
            tc.tile_pool(name="stage", bufs=3) as stage,
            tc.tile_pool(name="pg", bufs=2, space="PSUM") as pg,
            tc.tile_pool(name="pu", bufs=2, space="PSUM") as pu,
            tc.tile_pool(name="py", bufs=2, space="PSUM") as py,
            tc.tile_pool(name="dram", bufs=1, space="DRAM") as dram,
        ):
            # Resident weights, laid out [128, chunk, free] so that
            # [:, c, ts(j,128)] is a ready [K=128, M=128] stationary operand.
            wg_sb = wpool.tile([128, hc, i_sz], BF16, tag="wg")
            nc.sync.dma_start(out=wg_sb[:], in_=wg.rearrange("(c p) i -> p c i", p=128))
            wu_sb = wpool.tile([128, hc, i_sz], BF16, tag="wu")
            nc.sync.dma_start(out=wu_sb[:], in_=wu.rearrange("(c p) i -> p c i", p=128))
            wd_sb = wpool.tile([128, ic, h], BF16, tag="wd")
            nc.sync.dma_start(out=wd_sb[:], in_=wd.rearrange("(c p) j -> p c j", p=128))

            # Combine weights broadcast across all 128 partitions.
            wc_sb = wpool.tile([128, t], F32, tag="wc")
            nc.sync.dma_start(out=wc_sb[:], in_=wc.broadcast_to([128, t]))

            pT = dram.tile([ntb, h, tb], F32, tag="pT")
            rs = dram.tile([ntb, shard, tb], F32, tag="rs")

            for b in range(ntb):
                hs_t = hspool.tile([128, hc, tb], BF16)
                nc.sync.dma_start(
                    out=hs_t[:],
                    in_=hsT[:, ts(b, tb)].rearrange("(c p) t -> p c t", p=128))

                a_sb = apool.tile([128, ic, tb], BF16)
                for it in range(ic):
                    psg = pg.tile([128, tb], F32)
                    psu = pu.tile([128, tb], F32)
                    for c in range(hc):
                        nc.tensor.matmul(psg[:], lhsT=wg_sb[:, c, ts(it, 128)],
                                         rhs=hs_t[:, c, :],
                                         start=(c == 0), stop=(c == hc - 1))
                    for c in range(hc):
                        nc.tensor.matmul(psu[:], lhsT=wu_sb[:, c, ts(it, 128)],
                                         rhs=hs_t[:, c, :],
                                         start=(c == 0), stop=(c == hc - 1))
                    sil = stage.tile([128, tb], F32, tag="sil")
                    nc.scalar.activation(out=sil[:], in_=psg[:], func=silu)
                    nc.vector.tensor_mul(sil[:], sil[:], psg[:])
                    nc.vector.tensor_mul(a_sb[:, it, :], sil[:], psu[:])

                for ht in range(hc):
                    psy = py.tile([128, tb], F32)
                    for c2 in range(ic):
                        nc.tensor.matmul(psy[:], lhsT=wd_sb[:, c2, ts(ht, 128)],
                                         rhs=a_sb[:, c2, :],
                                         start=(c2 == 0), stop=(c2 == ic - 1))
                    po = stage.tile([128, tb], F32, tag="pout")
                    nc.vector.tensor_mul(po[:], psy[:], wc_sb[:, ts(b, tb)])
                    nc.sync.dma_start(out=pT[b, ts(ht, 128), :], in_=po[:])

                if use_collective:
                    nc.gpsimd.collective_compute(
                        "ReduceScatter", mybir.AluOpType.add,
                        replica_groups=rgroups,
                        ins=[pT[b].opt()], outs=[rs[b].opt()])
                    nc.sync.dma_start(out=out[b], in_=rs[b])
                else:
                    nc.sync.dma_start(out=out[b], in_=pT[b, :shard, :])

    nc.compile()
    return nc


def _build_sparse(h=H, i_sz=I, t=T, ncores=NCORES, chunk=2048, cd=640,
                  do_scatter=True, use_rs=True):
    """Sparse expert-parallel MoE. Core e receives only the tokens routed to
    expert e, gathered host-side into per-output-chunk segments of uniform
    capacity `cd` (so the program stays SPMD-identical on every core).
    Capacity C = nchunks*cd compact token slots. cd must be a multiple of
    128 so every 128-token scatter tile lies inside one segment — indirect
    DMA with a partition-offset source slice kills the device.

    Per-core inputs:
      hsTg [h, C]  bf16  gathered hidden states (transposed), pads = col 0
      wg/wu [h,i], wd [i,h]  bf16  expert weights
      wcg [C] f32   combine weights in compact order, pads = 0
      sidx [C] i32  chunk-local scatter row (t % chunk), pads = chunk
    Output: out [nchunks, chunk//ncores, h] bf16 — ReduceScatter shards.

    Compute: phase 1 produces aT = silu(g)*u in [i, tok] layout; phase 2
    computes y token-major (lhsT = aT tile), scales rows by wcg, and
    indirect-scatters 128-row tiles into per-chunk DRAM buffers which are
    ReduceScattered across cores as soon as their last write lands.
    """
    hc, ic2, nchunks = h // 128, i_sz // 128, t // chunk
    C = nchunks * cd
    assert cd % 128 == 0 and chunk % ncores == 0
    nct = C // 128
    HB = min(h, 512)
    nhb = h // HB
    shard = chunk // ncores

    # blocks of <=512 compact tokens for phase 1
    blocks = []
    pos = 0
    while pos < C:
        nb = min(512, C - pos)
        blocks.append((pos, nb))
        pos += nb

    nc = bacc.Bacc("TRN2", target_bir_lowering=False, debug=False,
                   num_devices=ncores)
    hsTg = nc.declare_dram_parameter("hsTg", [h, C], BF16, isOutput=False).ap()
    wg = nc.declare_dram_parameter("wg", [h, i_sz], BF16, isOutput=False).ap()
    wu = nc.declare_dram_parameter("wu", [h, i_sz], BF16, isOutput=False).ap()
    wd = nc.declare_dram_parameter("wd", [i_sz, h], BF16, isOutput=False).ap()
    wcg = nc.declare_dram_parameter("wcg", [C], F32, isOutput=False).ap()
    sidx = nc.declare_dram_parameter("sidx", [C], mybir.dt.int32,
                                     isOutput=False).ap()
    out = nc.declare_dram_parameter("out", [nchunks, shard, h], BF16,
                                    isOutput=True).ap()

    sigm = mybir.ActivationFunctionType.Sigmoid
    rgroups = [list(range(ncores))]

    with tile.TileContext(nc) as tc:
        with (
            tc.tile_pool(name="wpool", bufs=1) as wpool,
            tc.tile_pool(name="hspool", bufs=2) as hspool,
            tc.tile_pool(name="apool", bufs=1) as apool,
            tc.tile_pool(name="stage", bufs=3) as stage,
            tc.tile_pool(name="ypool", bufs=3) as ypool,
            tc.tile_pool(name="pg", bufs=2, space="PSUM") as pg,
            tc.tile_pool(name="pu", bufs=2, space="PSUM") as pu,
            tc.tile_pool(name="py", bufs=2, space="PSUM") as py,
            tc.tile_pool(name="dram", bufs=1, space="DRAM") as dram,
        ):
            wg_sb = wpool.tile([128, hc, i_sz], BF16, tag="wg")
            nc.sync.dma_start(out=wg_sb[:], in_=wg.rearrange("(c p) i -> p c i", p=128))
            wu_sb = wpool.tile([128, hc, i_sz], BF16, tag="wu")
            nc.sync.dma_start(out=wu_sb[:], in_=wu.rearrange("(c p) i -> p c i", p=128))
            wd_sb = wpool.tile([128, ic2, h], BF16, tag="wd")
            nc.sync.dma_start(out=wd_sb[:], in_=wd.rearrange("(c p) j -> p c j", p=128))
            wcg_sb = wpool.tile([128, nct], F32, tag="wcg")
            nc.sync.dma_start(out=wcg_sb[:], in_=wcg.rearrange("(ct p) -> p ct", p=128))
            sidx_sb = wpool.tile([128, nct], mybir.dt.int32, tag="sidx")
            nc.sync.dma_start(out=sidx_sb[:], in_=sidx.rearrange("(ct p) -> p ct", p=128))
            zsb = wpool.tile([128, h], BF16, tag="zero")
            nc.vector.memset(zsb[:], 0.0)

            pts = []
            for d in range(nchunks):
                pt_d = dram.tile([chunk + 128, h], BF16, name=f"pt{d}",
                                 tag=f"pt{d}")
                pts.append(pt_d)
                for q in range(chunk // 128):
                    nc.sync.dma_start(out=pt_d[ts(q, 128), :], in_=zsb[:])
            rs = dram.tile([nchunks, shard, h], BF16, tag="rs")

            for (pos, nb) in blocks:
                hs_t = hspool.tile([128, hc, nb], BF16, tag="hst")
                nc.sync.dma_start(
                    out=hs_t[:],
                    in_=hsTg[:, pos:pos + nb].rearrange("(c p) t -> p c t", p=128))

                aT = apool.tile([128, ic2, nb], BF16, tag="aT")
                for it in range(ic2):
                    psg = pg.tile([128, nb], F32, tag="psg")
                    psu = pu.tile([128, nb], F32, tag="psu")
                    for c in range(hc):
                        nc.tensor.matmul(psg[:], lhsT=wg_sb[:, c, ts(it, 128)],
                                         rhs=hs_t[:, c, :],
                                         start=(c == 0), stop=(c == hc - 1))
                    for c in range(hc):
                        nc.tensor.matmul(psu[:], lhsT=wu_sb[:, c, ts(it, 128)],
                                         rhs=hs_t[:, c, :],
                                         start=(c == 0), stop=(c == hc - 1))
                    sil = stage.tile([128, nb], F32, tag="sil")
                    nc.scalar.activation(out=sil[:], in_=psg[:], func=sigm)
                    nc.vector.tensor_mul(sil[:], sil[:], psg[:])
                    nc.vector.tensor_mul(aT[:, it, :], sil[:], psu[:])

                for ct in range(nb // 128):
                    gct = pos // 128 + ct
                    y_sb = ypool.tile([128, h], BF16, tag="ysb")
                    for hb in range(nhb):
                        psy = py.tile([128, HB], F32, tag="psy")
                        for c2 in range(ic2):
                            nc.tensor.matmul(psy[:],
                                             lhsT=aT[:, c2, ts(ct, 128)],
                                             rhs=wd_sb[:, c2, ts(hb, HB)],
                                             start=(c2 == 0),
                                             stop=(c2 == ic2 - 1))
                        nc.vector.tensor_scalar_mul(
                            y_sb[:, ts(hb, HB)], psy[:],
                            wcg_sb[:, gct:gct + 1])
                    d = gct * 128 // cd  # cd % 128 == 0 -> tile in one segment
                    if do_scatter:
                        nc.gpsimd.indirect_dma_start(
                            out=pts[d][:],
                            out_offset=bass.IndirectOffsetOnAxis(
                                ap=sidx_sb[:, gct:gct + 1], axis=0),
                            in_=y_sb[:],
                            in_offset=None)
                    else:
                        nc.sync.dma_start(
                            out=pts[d][ts(gct % (chunk // 128), 128), :],
                            in_=y_sb[:])

            for d in range(nchunks):
                if use_rs:
                    nc.gpsimd.collective_compute(
                        "ReduceScatter", mybir.AluOpType.add,
                        replica_groups=rgroups,
                        ins=[pts[d][:chunk, :].opt()], outs=[rs[d].opt()])
                    nc.sync.dma_start(out=out[d], in_=rs[d])
                else:
                    nc.sync.dma_start(out=out[d], in_=pts[d][:shard, :])

    nc.compile()
    return nc


class _Runner:
    """Compile once, execute many. Mirrors bass2jax.run_bass_via_pjrt's
    multi-core path but keeps the jitted callable (and device-resident
    inputs) alive so repeat executions skip XLA/NEFF compilation."""

    def __init__(self, nc, n_cores):
        from concourse import bass2jax
        from jax.experimental.shard_map import shard_map
        from jax.sharding import Mesh, PartitionSpec

        bass2jax.install_neuronx_cc_hook()
        assert nc.partition_id_tensor is None or True
        partition_name = (nc.partition_id_tensor.name
                          if nc.partition_id_tensor else None)

        in_names, out_names, out_avals, zero_outs = [], [], [], []
        for alloc in nc.m.functions[0].allocations:
            if not isinstance(alloc, mybir.MemoryLocationSet):
                continue
            name = alloc.memorylocations[0].name
            if alloc.kind == "ExternalInput":
                if name != partition_name:
                    in_names.append(name)
            elif alloc.kind == "ExternalOutput":
                shape = tuple(alloc.tensor_shape)
                dtype = mybir.dt.np(alloc.dtype)
                out_names.append(name)
                out_avals.append(jax.core.ShapedArray(shape, dtype))
                zero_outs.append(np.zeros(shape, dtype))
        self.n_params = len(in_names)
        self.param_names = list(in_names)
        self.out_names = out_names
        self.out_avals = out_avals
        self.n_cores = n_cores
        all_names = in_names + out_names
        if partition_name is not None:
            all_names.append(partition_name)

        def _body(*args):
            operands = list(args)
            if partition_name is not None:
                operands.append(bass2jax.partition_id_tensor())
            outs = bass2jax._bass_exec_p.bind(
                *operands,
                out_avals=tuple(out_avals),
                in_names=tuple(all_names),
                out_names=tuple(out_names),
                lowering_input_output_aliases=(),
                sim_require_finite=True,
                sim_require_nnan=True,
                nc=nc,
            )
            return tuple(outs)

        devices = jax.devices()[:n_cores]
        assert len(devices) == n_cores
        mesh = Mesh(np.asarray(devices), ("core",))
        n_ops = self.n_params + len(out_names)
        self._body = _body
        self._mesh = mesh
        self._in_specs = (PartitionSpec("core"),) * n_ops
        self._out_specs = (PartitionSpec("core"),) * len(out_names)
        self._shard_map = shard_map
        self._fn = jax.jit(
            shard_map(_body, mesh=mesh,
                      in_specs=self._in_specs,
                      out_specs=self._out_specs,
                      check_rep=False),
            keep_unused=True)
        self._chains = {}
        self._zeros = [
            np.zeros((n_cores * z.shape[0], *z.shape[1:]), z.dtype)
            for z in zero_outs
        ]
        self._dev_args = None

    def prepare(self, in_maps):
        """Stage concatenated inputs, sharded across cores so execution
        never reshards (resharding would ship bytes through the host)."""
        from jax.sharding import NamedSharding, PartitionSpec
        sh = NamedSharding(self._mesh, PartitionSpec("core"))
        concat = [
            np.concatenate([np.asarray(in_maps[c][name])
                            for c in range(self.n_cores)], axis=0)
            for name in self.param_names
        ]
        self._dev_args = [jax.device_put(a, sh) for a in concat + self._zeros]

    def execute(self):
        outs = self._fn(*self._dev_args)
        jax.block_until_ready(outs)
        return outs

    def execute_chain(self, k):
        """Issue k async executions back-to-back, block once at the end.
        Device-side queuing overlaps the per-dispatch host round-trip, so
        wall(k) ≈ floor + k * hw_exec and the slope over k isolates
        hw_exec."""
        outs = None
        for _ in range(k):
            outs = self._fn(*self._dev_args)
        jax.block_until_ready(outs)
        return outs

    def run(self, in_maps):
        self.prepare(in_maps)
        outs = self.execute()
        return [
            {name: np.asarray(outs[i]).reshape(self.n_cores,
                                               *self.out_avals[i].shape)[c]
             for i, name in enumerate(self.out_names)}
            for c in range(self.n_cores)
        ]


_RUNNER = None
_SPARSE_RUNNERS = {}


def _get_runner():
    global _RUNNER
    if _RUNNER is None:
        nc = _build_moe()
        _RUNNER = _Runner(nc, NCORES)
    return _RUNNER


def _get_sparse_runner(cd, chunk=2048):
    key = (cd, chunk)
    if key not in _SPARSE_RUNNERS:
        nc = _build_sparse(chunk=chunk, cd=cd)
        _SPARSE_RUNNERS[key] = _Runner(nc, NCORES)
    return _SPARSE_RUNNERS[key]


def _dispatch(hidden_states, top_k_index, top_k_weights, chunk=2048):
    """Host-side all-to-all dispatch: for each expert, gather its routed
    tokens grouped by output chunk, padded to a uniform per-chunk capacity
    cd (so all 8 cores run the identical SPMD program)."""
    hs = np.asarray(hidden_states, dtype=np.float32)
    idx = np.asarray(top_k_index).astype(np.int64)
    tw = np.asarray(top_k_weights, dtype=np.float32)
    nchunks = T // chunk

    w = np.zeros((E, T), dtype=np.float32)
    tarange = np.arange(T)
    for k in range(KTOP):
        np.add.at(w, (idx[:, k], tarange), tw[:, k])

    routed = np.zeros((E, T), dtype=bool)
    for k in range(KTOP):
        routed[idx[:, k], tarange] = True

    # counts per (expert, chunk) -> uniform capacity, 128-aligned
    counts = routed.reshape(E, nchunks, chunk).sum(axis=2)
    cd = max(128, int(-(-counts.max() // 128) * 128))
    C = nchunks * cd

    hsT_bf = np.ascontiguousarray(hs.T).astype(ml_dtypes.bfloat16)
    in_maps = []
    meta = []
    for e in range(E):
        cols = np.zeros(C, dtype=np.int64)  # source token per compact slot
        wcg = np.zeros(C, dtype=np.float32)
        sidx = np.full(C, chunk, dtype=np.int32)  # pads -> trash row
        for d in range(nchunks):
            toks = np.nonzero(routed[e, d * chunk:(d + 1) * chunk])[0] + d * chunk
            n = len(toks)
            base = d * cd
            cols[base:base + n] = toks
            wcg[base:base + n] = w[e, toks]
            sidx[base:base + n] = (toks % chunk).astype(np.int32)
        in_maps.append({
            "hsTg": np.ascontiguousarray(hsT_bf[:, cols]),
            "wg": None, "wu": None, "wd": None,  # filled by caller
            "wcg": wcg,
            "sidx": sidx,
        })
        meta.append(cols)
    return in_maps, cd


def _assemble_sparse(results, chunk=2048):
    nchunks, shard = T // chunk, chunk // NCORES
    full = np.empty((T, H), dtype=np.float32)
    for c in range(NCORES):
        r = np.asarray(results[c]["out"], dtype=np.float32)  # [nchunks, shard, H]
        for d in range(nchunks):
            full[d * chunk + c * shard: d * chunk + (c + 1) * shard, :] = r[d]
    return full


def _prep_in_maps(hidden_states, top_k_index, top_k_weights, Wg, Wu, Wd):
    hs = np.ascontiguousarray(np.asarray(hidden_states, dtype=np.float32))
    idx = np.asarray(top_k_index).astype(np.int64)
    tw = np.asarray(top_k_weights, dtype=np.float32)

    # Combine weights w[e,t] = sum_k tw[t,k] * [idx[t,k]==e]  (host: O(T*K))
    w = np.zeros((E, T), dtype=np.float32)
    tarange = np.arange(T)
    for k in range(KTOP):
        np.add.at(w, (idx[:, k], tarange), tw[:, k])

    hsT = np.ascontiguousarray(hs.T).astype(ml_dtypes.bfloat16)
    in_maps = []
    for e in range(NCORES):
        in_maps.append({
            "hsT": hsT,
            "wg": np.asarray(Wg[e], dtype=np.float32).astype(ml_dtypes.bfloat16),
            "wu": np.asarray(Wu[e], dtype=np.float32).astype(ml_dtypes.bfloat16),
            "wd": np.asarray(Wd[e], dtype=np.float32).astype(ml_dtypes.bfloat16),
            "wc": w[e:e + 1],
        })
    return in_maps


def _assemble(results):
    ntb, shard = T // TB, H // NCORES
    yT = np.empty((H, T), dtype=np.float32)
    for c in range(NCORES):
        r = results[c]["out"]  # [ntb, shard, TB]
        for b in range(ntb):
            yT[c * shard:(c + 1) * shard, b * TB:(b + 1) * TB] = r[b]
    return np.ascontiguousarray(yT.T)


def kernel(hidden_states, top_k_index, top_k_weights, Wg, Wu, Wd):
    in_maps, cd = _dispatch(hidden_states, top_k_index, top_k_weights)
    for e in range(E):
        in_maps[e]["wg"] = np.asarray(Wg[e], dtype=np.float32).astype(ml_dtypes.bfloat16)
        in_maps[e]["wu"] = np.asarray(Wu[e], dtype=np.float32).astype(ml_dtypes.bfloat16)
        in_maps[e]["wd"] = np.asarray(Wd[e], dtype=np.float32).astype(ml_dtypes.bfloat16)
    runner = _get_sparse_runner(cd)
    results = runner.run(in_maps)
    return _assemble_sparse(results)


def kernel_dense(hidden_states, top_k_index, top_k_weights, Wg, Wu, Wd):
    runner = _get_runner()
    in_maps = _prep_in_maps(hidden_states, top_k_index, top_k_weights,
                            Wg, Wu, Wd)
    results = runner.run(in_maps)
    return _assemble(results)


# revision 22
# speedup vs baseline: 98.2727x; 1.1995x over previous
"""MoE (top-2 of 8 experts, SwiGLU) kernel for 8 TRN2 NeuronCores.

Strategy: expert-parallel. Core e holds expert e's weights and computes the
dense masked formulation for ALL tokens: y_e = (silu(hs@Wg_e) * (hs@Wu_e)) @ Wd_e,
scaled per-token by the combine weight w[e,t] (zero if token t is not routed to
expert e). The weighted partials are summed across the 8 cores with per-block
ReduceScatter(add); each core ends with a 1/8 slice of the output.

Everything runs in "transposed land": the moving matmul operand is always the
token axis, so hidden_states is shipped as hsT=[H,T] and the kernel produces
yT=[H,T] shards; the host transposes back at unshard time.

Matmul operands are bf16 (fp32 PSUM accumulation) — rel err vs the fp32
reference is ~1e-3, far inside tolerance, and bf16 runs the PE at full rate.
"""

import numpy as np
import ml_dtypes

import jax
import concourse.bass as bass
import concourse.tile as tile
from concourse import bacc, mybir
from concourse.bass import ts

E, H, I, T, KTOP = 8, 2048, 1408, 4096, 2
NCORES = 8
TB = 512  # tokens per block (= max f32 PSUM free dim)

BF16 = mybir.dt.bfloat16
F32 = mybir.dt.float32


def _build_moe(h=H, i_sz=I, t=T, tb=TB, ncores=NCORES, use_collective=True):
    """Per-core SPMD graph. Inputs (per core e):
    hsT [h,t] bf16 (replicated), wg [h,i] bf16, wu [h,i] bf16, wd [i,h] bf16,
    wc [1,t] f32 (combine weights for this core's expert).
    Output: out [ntb, h//ncores, tb] f32 — this core's ReduceScatter shards.
    """
    hc, ic, ntb = h // 128, i_sz // 128, t // tb
    shard = h // ncores
    nc = bacc.Bacc("TRN2", target_bir_lowering=False, debug=False,
                   num_devices=ncores)

    hsT = nc.declare_dram_parameter("hsT", [h, t], BF16, isOutput=False).ap()
    wg = nc.declare_dram_parameter("wg", [h, i_sz], BF16, isOutput=False).ap()
    wu = nc.declare_dram_parameter("wu", [h, i_sz], BF16, isOutput=False).ap()
    wd = nc.declare_dram_parameter("wd", [i_sz, h], BF16, isOutput=False).ap()
    wc = nc.declare_dram_parameter("wc", [1, t], F32, isOutput=False).ap()
    out = nc.declare_dram_parameter("out", [ntb, shard, tb], F32,
                                    isOutput=True).ap()

    silu = mybir.ActivationFunctionType.Sigmoid
    rgroups = [list(range(ncores))]

    with tile.TileContext(nc) as tc:
        with (
            tc.tile_pool(name="wpool", bufs=1) as wpool,
            tc.tile_pool(name="hspool", bufs=2) as hspool,
            tc.tile_pool(name="apool", bufs=1) as apool,
            tc.tile_pool(name="stage", bufs=3) as stage,
            tc.tile_pool(name="pg", bufs=2, space="PSUM") as pg,
            tc.tile_pool(name="pu", bufs=2, space="PSUM") as pu,
            tc.tile_pool(name="py", bufs=2, space="PSUM") as py,
            tc.tile_pool(name="dram", bufs=1, space="DRAM") as dram,
        ):
            # Resident weights, laid out [128, chunk, free] so that
            # [:, c, ts(j,128)] is a ready [K=128, M=128] stationary operand.
            wg_sb = wpool.tile([128, hc, i_sz], BF16, tag="wg")
            nc.sync.dma_start(out=wg_sb[:], in_=wg.rearrange("(c p) i -> p c i", p=128))
            wu_sb = wpool.tile([128, hc, i_sz], BF16, tag="wu")
            nc.sync.dma_start(out=wu_sb[:], in_=wu.rearrange("(c p) i -> p c i", p=128))
            wd_sb = wpool.tile([128, ic, h], BF16, tag="wd")
            nc.sync.dma_start(out=wd_sb[:], in_=wd.rearrange("(c p) j -> p c j", p=128))

            # Combine weights broadcast across all 128 partitions.
            wc_sb = wpool.tile([128, t], F32, tag="wc")
            nc.sync.dma_start(out=wc_sb[:], in_=wc.broadcast_to([128, t]))

            pT = dram.tile([ntb, h, tb], F32, tag="pT")
            rs = dram.tile([ntb, shard, tb], F32, tag="rs")

            for b in range(ntb):
                hs_t = hspool.tile([128, hc, tb], BF16)
                nc.sync.dma_start(
                    out=hs_t[:],
                    in_=hsT[:, ts(b, tb)].rearrange("(c p) t -> p c t", p=128))

                a_sb = apool.tile([128, ic, tb], BF16)
                for it in range(ic):
                    psg = pg.tile([128, tb], F32)
                    psu = pu.tile([128, tb], F32)
                    for c in range(hc):
                        nc.tensor.matmul(psg[:], lhsT=wg_sb[:, c, ts(it, 128)],
                                         rhs=hs_t[:, c, :],
                                         start=(c == 0), stop=(c == hc - 1))
                    for c in range(hc):
                        nc.tensor.matmul(psu[:], lhsT=wu_sb[:, c, ts(it, 128)],
                                         rhs=hs_t[:, c, :],
                                         start=(c == 0), stop=(c == hc - 1))
                    sil = stage.tile([128, tb], F32, tag="sil")
                    nc.scalar.activation(out=sil[:], in_=psg[:], func=silu)
                    nc.vector.tensor_mul(sil[:], sil[:], psg[:])
                    nc.vector.tensor_mul(a_sb[:, it, :], sil[:], psu[:])

                for ht in range(hc):
                    psy = py.tile([128, tb], F32)
                    for c2 in range(ic):
                        nc.tensor.matmul(psy[:], lhsT=wd_sb[:, c2, ts(ht, 128)],
                                         rhs=a_sb[:, c2, :],
                                         start=(c2 == 0), stop=(c2 == ic - 1))
                    po = stage.tile([128, tb], F32, tag="pout")
                    nc.vector.tensor_mul(po[:], psy[:], wc_sb[:, ts(b, tb)])
                    nc.sync.dma_start(out=pT[b, ts(ht, 128), :], in_=po[:])

                if use_collective:
                    nc.gpsimd.collective_compute(
                        "ReduceScatter", mybir.AluOpType.add,
                        replica_groups=rgroups,
                        ins=[pT[b].opt()], outs=[rs[b].opt()])
                    nc.sync.dma_start(out=out[b], in_=rs[b])
                else:
                    nc.sync.dma_start(out=out[b], in_=pT[b, :shard, :])

    nc.compile()
    return nc


def _build_sparse(h=H, i_sz=I, t=T, ncores=NCORES, chunk=2048, cd=640,
                  do_scatter=True, use_rs=True):
    """Sparse expert-parallel MoE. Core e receives only the tokens routed to
    expert e, gathered host-side into per-output-chunk segments of uniform
    capacity `cd` (so the program stays SPMD-identical on every core).
    Capacity C = nchunks*cd compact token slots. cd must be a multiple of
    128 so every 128-token scatter tile lies inside one segment — indirect
    DMA with a partition-offset source slice kills the device.

    Per-core inputs:
      hsTg [h, C]  bf16  gathered hidden states (transposed), pads = col 0
      wg/wu [h,i], wd [i,h]  bf16  expert weights
      wcg [C] f32   combine weights in compact order, pads = 0
      sidx [C] i32  chunk-local scatter row (t % chunk), pads = chunk
    Output: out [nchunks, chunk//ncores, h] bf16 — ReduceScatter shards.

    Compute: phase 1 produces aT = silu(g)*u in [i, tok] layout; phase 2
    computes y token-major (lhsT = aT tile), scales rows by wcg, and
    indirect-scatters 128-row tiles into per-chunk DRAM buffers which are
    ReduceScattered across cores as soon as their last write lands.
    """
    hc, ic2, nchunks = h // 128, i_sz // 128, t // chunk
    C = nchunks * cd
    assert cd % 128 == 0 and chunk % ncores == 0
    nct = C // 128
    HB = min(h, 512)
    nhb = h // HB
    shard = chunk // ncores

    # blocks of <=512 compact tokens for phase 1
    blocks = []
    pos = 0
    while pos < C:
        nb = min(512, C - pos)
        blocks.append((pos, nb))
        pos += nb

    nc = bacc.Bacc("TRN2", target_bir_lowering=False, debug=False,
                   num_devices=ncores)
    hsTg = nc.declare_dram_parameter("hsTg", [h, C], BF16, isOutput=False).ap()
    wg = nc.declare_dram_parameter("wg", [h, i_sz], BF16, isOutput=False).ap()
    wu = nc.declare_dram_parameter("wu", [h, i_sz], BF16, isOutput=False).ap()
    wd = nc.declare_dram_parameter("wd", [i_sz, h], BF16, isOutput=False).ap()
    wcg = nc.declare_dram_parameter("wcg", [C], F32, isOutput=False).ap()
    sidx = nc.declare_dram_parameter("sidx", [C], mybir.dt.int32,
                                     isOutput=False).ap()
    out = nc.declare_dram_parameter("out", [nchunks, shard, h], BF16,
                                    isOutput=True).ap()

    sigm = mybir.ActivationFunctionType.Sigmoid
    rgroups = [list(range(ncores))]

    with tile.TileContext(nc) as tc:
        with (
            tc.tile_pool(name="wpool", bufs=1) as wpool,
            tc.tile_pool(name="hspool", bufs=2) as hspool,
            tc.tile_pool(name="apool", bufs=1) as apool,
            tc.tile_pool(name="stage", bufs=3) as stage,
            tc.tile_pool(name="ypool", bufs=3) as ypool,
            tc.tile_pool(name="pg", bufs=2, space="PSUM") as pg,
            tc.tile_pool(name="pu", bufs=2, space="PSUM") as pu,
            tc.tile_pool(name="py", bufs=2, space="PSUM") as py,
            tc.tile_pool(name="dram", bufs=1, space="DRAM") as dram,
        ):
            wg_sb = wpool.tile([128, hc, i_sz], BF16, tag="wg")
            nc.sync.dma_start(out=wg_sb[:], in_=wg.rearrange("(c p) i -> p c i", p=128))
            wu_sb = wpool.tile([128, hc, i_sz], BF16, tag="wu")
            nc.sync.dma_start(out=wu_sb[:], in_=wu.rearrange("(c p) i -> p c i", p=128))
            wd_sb = wpool.tile([128, ic2, h], BF16, tag="wd")
            nc.sync.dma_start(out=wd_sb[:], in_=wd.rearrange("(c p) j -> p c j", p=128))
            wcg_sb = wpool.tile([128, nct], F32, tag="wcg")
            nc.sync.dma_start(out=wcg_sb[:], in_=wcg.rearrange("(ct p) -> p ct", p=128))
            sidx_sb = wpool.tile([128, nct], mybir.dt.int32, tag="sidx")
            nc.sync.dma_start(out=sidx_sb[:], in_=sidx.rearrange("(ct p) -> p ct", p=128))
            zsb = wpool.tile([128, h], BF16, tag="zero")
            nc.vector.memset(zsb[:], 0.0)

            pts = []
            for d in range(nchunks):
                pt_d = dram.tile([chunk + 128, h], BF16, name=f"pt{d}",
                                 tag=f"pt{d}")
                pts.append(pt_d)
                for q in range(chunk // 128):
                    nc.sync.dma_start(out=pt_d[ts(q, 128), :], in_=zsb[:])
            rs = dram.tile([nchunks, shard, h], BF16, tag="rs")

            for (pos, nb) in blocks:
                hs_t = hspool.tile([128, hc, nb], BF16, tag="hst")
                nc.sync.dma_start(
                    out=hs_t[:],
                    in_=hsTg[:, pos:pos + nb].rearrange("(c p) t -> p c t", p=128))

                aT = apool.tile([128, ic2, nb], BF16, tag="aT")
                for it in range(ic2):
                    psg = pg.tile([128, nb], F32, tag="psg")
                    psu = pu.tile([128, nb], F32, tag="psu")
                    for c in range(hc):
                        nc.tensor.matmul(psg[:], lhsT=wg_sb[:, c, ts(it, 128)],
                                         rhs=hs_t[:, c, :],
                                         start=(c == 0), stop=(c == hc - 1))
                    for c in range(hc):
                        nc.tensor.matmul(psu[:], lhsT=wu_sb[:, c, ts(it, 128)],
                                         rhs=hs_t[:, c, :],
                                         start=(c == 0), stop=(c == hc - 1))
                    sil = stage.tile([128, nb], F32, tag="sil")
                    nc.scalar.activation(out=sil[:], in_=psg[:], func=sigm)
                    nc.vector.tensor_mul(sil[:], sil[:], psg[:])
                    nc.vector.tensor_mul(aT[:, it, :], sil[:], psu[:])

                for ct in range(nb // 128):
                    gct = pos // 128 + ct
                    y_sb = ypool.tile([128, h], BF16, tag="ysb")
                    for hb in range(nhb):
                        psy = py.tile([128, HB], F32, tag="psy")
                        for c2 in range(ic2):
                            nc.tensor.matmul(psy[:],
                                             lhsT=aT[:, c2, ts(ct, 128)],
                                             rhs=wd_sb[:, c2, ts(hb, HB)],
                                             start=(c2 == 0),
                                             stop=(c2 == ic2 - 1))
                        nc.vector.tensor_scalar_mul(
                            y_sb[:, ts(hb, HB)], psy[:],
                            wcg_sb[:, gct:gct + 1])
                    d = gct * 128 // cd  # cd % 128 == 0 -> tile in one segment
                    if do_scatter:
                        nc.gpsimd.indirect_dma_start(
                            out=pts[d][:],
                            out_offset=bass.IndirectOffsetOnAxis(
                                ap=sidx_sb[:, gct:gct + 1], axis=0),
                            in_=y_sb[:],
                            in_offset=None)
                    else:
                        nc.sync.dma_start(
                            out=pts[d][ts(gct % (chunk // 128), 128), :],
                            in_=y_sb[:])

            for d in range(nchunks):
                if use_rs:
                    nc.gpsimd.collective_compute(
                        "ReduceScatter", mybir.AluOpType.add,
                        replica_groups=rgroups,
                        ins=[pts[d][:chunk, :].opt()], outs=[rs[d].opt()])
                    nc.sync.dma_start(out=out[d], in_=rs[d])
                else:
                    nc.sync.dma_start(out=out[d], in_=pts[d][:shard, :])

    nc.compile()
    return nc


class _Runner:
    """Compile once, execute many. Mirrors bass2jax.run_bass_via_pjrt's
    multi-core path but keeps the jitted callable (and device-resident
    inputs) alive so repeat executions skip XLA/NEFF compilation."""

    def __init__(self, nc, n_cores):
        from concourse import bass2jax
        from jax.experimental.shard_map import shard_map
        from jax.sharding import Mesh, PartitionSpec

        bass2jax.install_neuronx_cc_hook()
        assert nc.partition_id_tensor is None or True
        partition_name = (nc.partition_id_tensor.name
                          if nc.partition_id_tensor else None)

        in_names, out_names, out_avals, zero_outs = [], [], [], []
        for alloc in nc.m.functions[0].allocations:
            if not isinstance(alloc, mybir.MemoryLocationSet):
                continue
            name = alloc.memorylocations[0].name
            if alloc.kind == "ExternalInput":
                if name != partition_name:
                    in_names.append(name)
            elif alloc.kind == "ExternalOutput":
                shape = tuple(alloc.tensor_shape)
                dtype = mybir.dt.np(alloc.dtype)
                out_names.append(name)
                out_avals.append(jax.core.ShapedArray(shape, dtype))
                zero_outs.append(np.zeros(shape, dtype))
        self.n_params = len(in_names)
        self.param_names = list(in_names)
        self.out_names = out_names
        self.out_avals = out_avals
        self.n_cores = n_cores
        all_names = in_names + out_names
        if partition_name is not None:
            all_names.append(partition_name)

        def _body(*args):
            operands = list(args)
            if partition_name is not None:
                operands.append(bass2jax.partition_id_tensor())
            outs = bass2jax._bass_exec_p.bind(
                *operands,
                out_avals=tuple(out_avals),
                in_names=tuple(all_names),
                out_names=tuple(out_names),
                lowering_input_output_aliases=(),
                sim_require_finite=True,
                sim_require_nnan=True,
                nc=nc,
            )
            return tuple(outs)

        devices = jax.devices()[:n_cores]
        assert len(devices) == n_cores
        mesh = Mesh(np.asarray(devices), ("core",))
        n_ops = self.n_params + len(out_names)
        self._body = _body
        self._mesh = mesh
        self._in_specs = (PartitionSpec("core"),) * n_ops
        self._out_specs = (PartitionSpec("core"),) * len(out_names)
        self._shard_map = shard_map
        self._fn = jax.jit(
            shard_map(_body, mesh=mesh,
                      in_specs=self._in_specs,
                      out_specs=self._out_specs,
                      check_rep=False),
            keep_unused=True)
        self._chains = {}
        self._zeros = [
            np.zeros((n_cores * z.shape[0], *z.shape[1:]), z.dtype)
            for z in zero_outs
        ]
        self._dev_args = None

    def prepare(self, in_maps):
        """Stage concatenated inputs, sharded across cores so execution
        never reshards (resharding would ship bytes through the host)."""
        from jax.sharding import NamedSharding, PartitionSpec
        sh = NamedSharding(self._mesh, PartitionSpec("core"))
        concat = [
            np.concatenate([np.asarray(in_maps[c][name])
                            for c in range(self.n_cores)], axis=0)
            for name in self.param_names
        ]
        self._dev_args = [jax.device_put(a, sh) for a in concat + self._zeros]

    def execute(self):
        outs = self._fn(*self._dev_args)
        jax.block_until_ready(outs)
        return outs

    def execute_chain(self, k):
        """Issue k async executions back-to-back, block once at the end.
        Device-side queuing overlaps the per-dispatch host round-trip, so
        wall(k) ≈ floor + k * hw_exec and the slope over k isolates
        hw_exec."""
        outs = None
        for _ in range(k):
            outs = self._fn(*self._dev_args)
        jax.block_until_ready(outs)
        return outs

    def run(self, in_maps):
        self.prepare(in_maps)
        outs = self.execute()
        return [
            {name: np.asarray(outs[i]).reshape(self.n_cores,
                                               *self.out_avals[i].shape)[c]
             for i, name in enumerate(self.out_names)}
            for c in range(self.n_cores)
        ]


_RUNNER = None
_SPARSE_RUNNERS = {}


def _get_runner():
    global _RUNNER
    if _RUNNER is None:
        nc = _build_moe()
        _RUNNER = _Runner(nc, NCORES)
    return _RUNNER


def _get_sparse_runner(cd, chunk=2048):
    key = (cd, chunk)
    if key not in _SPARSE_RUNNERS:
        nc = _build_sparse(chunk=chunk, cd=cd)
        _SPARSE_RUNNERS[key] = _Runner(nc, NCORES)
    return _SPARSE_RUNNERS[key]


def _dispatch(hidden_states, top_k_index, top_k_weights, chunk=2048):
    """Host-side all-to-all dispatch: for each expert, gather its routed
    tokens grouped by output chunk, padded to a uniform per-chunk capacity
    cd (so all 8 cores run the identical SPMD program)."""
    hs = np.asarray(hidden_states, dtype=np.float32)
    idx = np.asarray(top_k_index).astype(np.int64)
    tw = np.asarray(top_k_weights, dtype=np.float32)
    nchunks = T // chunk

    w = np.zeros((E, T), dtype=np.float32)
    tarange = np.arange(T)
    for k in range(KTOP):
        np.add.at(w, (idx[:, k], tarange), tw[:, k])

    routed = np.zeros((E, T), dtype=bool)
    for k in range(KTOP):
        routed[idx[:, k], tarange] = True

    # counts per (expert, chunk) -> uniform capacity, 128-aligned
    counts = routed.reshape(E, nchunks, chunk).sum(axis=2)
    cd = max(128, int(-(-counts.max() // 128) * 128))
    C = nchunks * cd
    print(f"[dispatch] chunk={chunk} max_count={counts.max()} cd={cd} C={C}")

    hsT_bf = np.ascontiguousarray(hs.T).astype(ml_dtypes.bfloat16)
    in_maps = []
    meta = []
    for e in range(E):
        cols = np.zeros(C, dtype=np.int64)  # source token per compact slot
        wcg = np.zeros(C, dtype=np.float32)
        sidx = np.full(C, chunk, dtype=np.int32)  # pads -> trash row
        for d in range(nchunks):
            toks = np.nonzero(routed[e, d * chunk:(d + 1) * chunk])[0] + d * chunk
            n = len(toks)
            base = d * cd
            cols[base:base + n] = toks
            wcg[base:base + n] = w[e, toks]
            sidx[base:base + n] = (toks % chunk).astype(np.int32)
        in_maps.append({
            "hsTg": np.ascontiguousarray(hsT_bf[:, cols]),
            "wg": None, "wu": None, "wd": None,  # filled by caller
            "wcg": wcg,
            "sidx": sidx,
        })
        meta.append(cols)
    return in_maps, cd


def _assemble_sparse(results, chunk=2048):
    nchunks, shard = T // chunk, chunk // NCORES
    full = np.empty((T, H), dtype=np.float32)
    for c in range(NCORES):
        r = np.asarray(results[c]["out"], dtype=np.float32)  # [nchunks, shard, H]
        for d in range(nchunks):
            full[d * chunk + c * shard: d * chunk + (c + 1) * shard, :] = r[d]
    return full


def _prep_in_maps(hidden_states, top_k_index, top_k_weights, Wg, Wu, Wd):
    hs = np.ascontiguousarray(np.asarray(hidden_states, dtype=np.float32))
    idx = np.asarray(top_k_index).astype(np.int64)
    tw = np.asarray(top_k_weights, dtype=np.float32)

    # Combine weights w[e,t] = sum_k tw[t,k] * [idx[t,k]==e]  (host: O(T*K))
    w = np.zeros((E, T), dtype=np.float32)
    tarange = np.arange(T)
    for k in range(KTOP):
        np.add.at(w, (idx[:, k], tarange), tw[:, k])

    hsT = np.ascontiguousarray(hs.T).astype(ml_dtypes.bfloat16)
    in_maps = []
    for e in range(NCORES):
        in_maps.append({
            "hsT": hsT,
            "wg": np.asarray(Wg[e], dtype=np.float32).astype(ml_dtypes.bfloat16),
            "wu": np.asarray(Wu[e], dtype=np.float32).astype(ml_dtypes.bfloat16),
            "wd": np.asarray(Wd[e], dtype=np.float32).astype(ml_dtypes.bfloat16),
            "wc": w[e:e + 1],
        })
    return in_maps


def _assemble(results):
    ntb, shard = T // TB, H // NCORES
    yT = np.empty((H, T), dtype=np.float32)
    for c in range(NCORES):
        r = results[c]["out"]  # [ntb, shard, TB]
        for b in range(ntb):
            yT[c * shard:(c + 1) * shard, b * TB:(b + 1) * TB] = r[b]
    return np.ascontiguousarray(yT.T)


def kernel(hidden_states, top_k_index, top_k_weights, Wg, Wu, Wd):
    in_maps, cd = _dispatch(hidden_states, top_k_index, top_k_weights)
    for e in range(E):
        in_maps[e]["wg"] = np.asarray(Wg[e], dtype=np.float32).astype(ml_dtypes.bfloat16)
        in_maps[e]["wu"] = np.asarray(Wu[e], dtype=np.float32).astype(ml_dtypes.bfloat16)
        in_maps[e]["wd"] = np.asarray(Wd[e], dtype=np.float32).astype(ml_dtypes.bfloat16)
    runner = _get_sparse_runner(cd)
    results = runner.run(in_maps)
    return _assemble_sparse(results)


def kernel_dense(hidden_states, top_k_index, top_k_weights, Wg, Wu, Wd):
    runner = _get_runner()
    in_maps = _prep_in_maps(hidden_states, top_k_index, top_k_weights,
                            Wg, Wu, Wd)
    results = runner.run(in_maps)
    return _assemble(results)
